# revision 1
# baseline (speedup 1.0000x reference)
"""Trainium2 Bass kernel for nn_NodeFeatures (GNN message passing).

Math (per batch b):
    Ux  = (x @ U_w.T + U_b) * 0.5                      # (N, H)
    Vx  = (x @ V_w.T + V_b) * 0.5                      # (N, H)
    agg[i,h]   = sum_j gate[i,j,h] * Vx[j,h]
    denom[i,h] = 1e-20 + sum_j gate[i,j,h]
    out = Ux + agg / denom

Sharding: data-parallel over batch B=8 across the 8 NeuronCores (one batch
per core); H x H weights replicated.

Per-core plan (memory-bound: 32MB of gate streamed once, ~90us roofline;
TimelineSim cost model: ~111us):
  - gate tiles [j=128, i16=16, h=128] DMA'd with f32->bf16 cast (SWDGE).
  - DVE: prod = gate * Vx (bf16 2x-mode, one pass, ~36us).
  - PE: ones-column matmuls (lhsT=[128,1], bf16, N=512) reduce over j for
    both prod (-> agg) and gate (-> denom); each [1,512] row lands at
    partition 32*c of a [128,1024] 2-bank PSUM tile (tile_position col
    strips), accumulated over the two j-halves via start/stop (~55us).
  - Drains: [1,1024] ACT/DVE copies per strip to a stage tensor (rows at
    partitions 0/32/64/96); compute engines cannot address non-contiguous
    partitions, so one copy per strip.
  - Epilogue: flatten-order DMAs repack stage rows to [64,1024] (partition
    g' = 16c+g2 holds nodes i=16*g2+4c+q); Ux arrives in the same layout
    via a DRAM round-trip; DVE computes Ux + agg * recip(denom); one
    contiguous-per-partition DMA writes the output.
  - The last tile runs per-quarter (DMA/TT/matmul/drain) on two separate
    PSUM tiles so its drains overlap the stream tail on both ACT and DVE.
"""

import sys

import numpy as np

try:
    import concourse.bass as bass  # noqa: F401
except ImportError:  # pragma: no cover
    sys.path.insert(0, "/opt/trn_rl_repo")

from contextlib import ExitStack

import concourse.bacc as bacc
import concourse.mybir as mybir
import concourse.tile as tile
from concourse import bass_utils
from concourse.masks import make_identity

F32 = mybir.dt.float32
BF16 = mybir.dt.bfloat16

B, N, H = 8, 256, 128
NCORES = 8
I16 = 16              # i values per gate tile
G2 = N // I16         # 16 tiles per j-half
JT = N // 128         # 2 j-halves

# Config knobs (validated in sim; flip if a path misbehaves on HW)
GATE_BF16 = True      # cast gate f32->bf16 during DMA (SWDGE)
RECIP_APPROX = False  # custom-DVE op crashes under the axon compile path
ACT_DMA = True        # issue some epilogue DMAs on nc.scalar (HWDGE via ACT)


def build_program():
    """Build the per-core Bass program (identical on all 8 cores)."""
    nc = bacc.Bacc("TRN2", target_bir_lowering=False, debug=False,
                   num_devices=NCORES)

    x_d = nc.dram_tensor("x", [N, H], F32, kind="ExternalInput").ap()
    g_d = nc.dram_tensor("gate", [N, N, H], F32, kind="ExternalInput").ap()
    uw_d = nc.dram_tensor("U_w", [H, H], F32, kind="ExternalInput").ap()
    ub_d = nc.dram_tensor("U_b", [H], F32, kind="ExternalInput").ap()
    vw_d = nc.dram_tensor("V_w", [H, H], F32, kind="ExternalInput").ap()
    vb_d = nc.dram_tensor("V_b", [H], F32, kind="ExternalInput").ap()
    out_d = nc.dram_tensor("out", [N, H], F32, kind="ExternalOutput").ap()

    gate_dt = BF16 if GATE_BF16 else F32

    with tile.TileContext(nc) as tc, ExitStack() as ctx:
        const = ctx.enter_context(tc.tile_pool(name="const", bufs=1))

        # ---- small input loads -------------------------------------------
        x_sb = const.tile([128, 2, H], F32)           # [i_in_block, blk, h]
        nc.sync.dma_start(x_sb, x_d.rearrange("(b i) h -> i b h", i=128))
        uw_sb = const.tile([H, H], F32)
        nc.sync.dma_start(uw_sb, uw_d)
        vw_sb = const.tile([H, H], F32)
        nc.sync.dma_start(vw_sb, vw_d)
        # bias rows broadcast to all partitions (0-stride DRAM src), then *0.5
        bu_half = const.tile([128, H], F32)
        nc.sync.dma_start(bu_half, ub_d[None, :].to_broadcast((128, H)))
        nc.vector.tensor_scalar_mul(bu_half, bu_half, 0.5)
        bv_half = const.tile([128, H], F32)
        nc.sync.dma_start(bv_half, vb_d[None, :].to_broadcast((128, H)))
        nc.vector.tensor_scalar_mul(bv_half, bv_half, 0.5)

        ident = const.tile([128, 128], F32)
        make_identity(nc, ident)
        ones_col = const.tile([128, 1], gate_dt)
        nc.gpsimd.memset(ones_col, 1.0)

        # ---- setup: transposes and Ux/Vx ---------------------------------
        xT = const.tile([H, N], F32)                  # [h, i]
        uwT = const.tile([H, H], F32)                 # [h, k]
        vwT = const.tile([H, H], F32)
        ux_sb = const.tile([128, 2, H], F32)          # [i_in_block, blk, h]
        vx0 = const.tile([128, 1, 1, H], BF16)        # [j, 1, 1, h] for j-half 0
        vx1 = const.tile([128, 1, 1, H], BF16)
        vx = [vx0, vx1]

        with tc.tile_pool(name="spsum", bufs=2, space="PSUM") as spsum:
            for blk in range(2):
                pt = spsum.tile([128, 128], F32, tag="tr")
                nc.tensor.transpose(pt, x_sb[:, blk, :], ident)
                nc.scalar.copy(xT[:, blk * 128:(blk + 1) * 128], pt)
            ptu = spsum.tile([128, 128], F32, tag="tr")
            nc.tensor.transpose(ptu, uw_sb, ident)
            nc.scalar.copy(uwT, ptu)
            ptv = spsum.tile([128, 128], F32, tag="tr")
            nc.tensor.transpose(ptv, vw_sb, ident)
            nc.scalar.copy(vwT, ptv)

            for blk in range(2):
                lhs = xT[:, blk * 128:(blk + 1) * 128]
                pv = spsum.tile([128, 128], F32, tag="mm")
                nc.tensor.matmul(pv, lhsT=lhs, rhs=vwT, start=True, stop=True)
                # vx = psum*0.5 + 0.5*V_b  (cast to bf16 on write)
                nc.vector.scalar_tensor_tensor(
                    vx[blk][:, 0, 0, :], pv, 0.5, bv_half,
                    op0=mybir.AluOpType.mult, op1=mybir.AluOpType.add)
                pu = spsum.tile([128, 128], F32, tag="mm")
                nc.tensor.matmul(pu, lhsT=lhs, rhs=uwT, start=True, stop=True)
                nc.vector.scalar_tensor_tensor(
                    ux_sb[:, blk, :], pu, 0.5, bu_half,
                    op0=mybir.AluOpType.mult, op1=mybir.AluOpType.add)

        # ---- main stream over gate ---------------------------------------
        # DRAM view: [g2, jt, j, i16, h]
        gv = g_d.rearrange("(g i) (t j) h -> g t j i h", i=I16, j=128)

        # agg|denom rows interleaved: partition 32c, free (g2, a/d, q, h)
        stage_ad = const.tile([128, G2 * 1024], F32)

        gate_pool = ctx.enter_context(tc.tile_pool(name="gate", bufs=4))
        prod_pool = ctx.enter_context(tc.tile_pool(name="prod", bufs=3))
        mpsum = ctx.enter_context(tc.tile_pool(name="mpsum", bufs=3, space="PSUM"))

        dma_cast = nc.gpsimd.dma_start if GATE_BF16 else nc.sync.dma_start

        def issue_gate_dma(g2, jt):
            gt = gate_pool.tile([128, I16, H], gate_dt, tag="g",
                                name=f"gt_{g2}_{jt}")
            dma_cast(gt, gv[g2, jt])
            return gt

        # front-run the first tile's DMAs so the stream starts at t=0
        pre = {(0, jt): issue_gate_dma(0, jt) for jt in range(JT)}

        for g2 in range(G2):
            last = g2 == G2 - 1
            # one 2-bank psum tile per g2: agg in [:, :512], denom in
            # [:, 512:].  The last g2 uses two tiles (different banks) so
            # its final drains run on ACT and DVE in parallel.
            if last:
                acc_e = mpsum.tile([128, 1024], F32, tag="AD", name="acc_le")
                acc_o = mpsum.tile([128, 1024], F32, tag="AD", name="acc_lo")
                accs = [acc_e, acc_o]
            else:
                a = mpsum.tile([128, 1024], F32, tag="AD", name=f"acc_{g2}")
                accs = [a, a]
            sl = slice(g2 * 1024, (g2 + 1) * 1024)

            def drain(c):
                pp = slice(32 * c, 32 * c + 1)
                acc = accs[c // 2]
                # acc_e strips (c<2) drain on ACT, acc_o strips on DVE;
                # mid-stream only strip 3 goes to DVE
                on_dve = (c >= 2) if last else (c == 3)
                if on_dve:
                    nc.vector.tensor_copy(stage_ad[pp, sl], acc[pp, :])
                else:
                    nc.scalar.copy(stage_ad[pp, sl], acc[pp, :])

            for jt in range(JT):
                split = last
                if split:
                    # final tile: per-quarter DMA/TT/matmuls so the early
                    # quarters' compute (and per-strip-pair drains) overlap
                    # the tail of the stream
                    gt = gate_pool.tile([128, I16, H], gate_dt, tag="g",
                                        name=f"gt_{g2}_{jt}")
                else:
                    gt = pre.pop((g2, jt), None)
                    if gt is None:
                        gt = issue_gate_dma(g2, jt)
                pr = prod_pool.tile([128, I16, H], gate_dt, tag="p",
                                    name=f"pr_{g2}_{jt}")
                if not split:
                    nc.vector.tensor_mul(
                        pr, gt, vx[jt][:, 0].to_broadcast((128, I16, H)))
                for n4 in range(4):
                    q = slice(4 * n4, 4 * n4 + 4)
                    if split:
                        dma_cast(gt[:, q, :], gv[g2, jt, :, q, :])
                        nc.vector.tensor_mul(
                            pr[:, q, :], gt[:, q, :],
                            vx[jt][:, 0].to_broadcast((128, 4, H)))
                    acc = accs[n4 // 2]
                    o_a = acc[32 * n4:32 * n4 + 1, 0:512]
                    o_d = acc[32 * n4:32 * n4 + 1, 512:1024]
                    tp = (0, 32 * n4)
                    nc.tensor.matmul(o_a, lhsT=ones_col, rhs=pr[:, q, :],
                                     start=(jt == 0), stop=(jt == JT - 1),
                                     tile_position=tp)
                    nc.tensor.matmul(o_d, lhsT=ones_col, rhs=gt[:, q, :],
                                     start=(jt == 0), stop=(jt == JT - 1),
                                     tile_position=tp)
                    if last and jt == JT - 1 and n4 % 2 == 1:
                        # strip pair complete: drain it while PE moves on to
                        # the other acc tile
                        drain(n4 - 1)
                        drain(n4)
            if not last:
                for c in range(4):
                    drain(c)

        # ---- epilogue -----------------------------------------------------
        # Pack stage rows -> [64, 1024].  Partition g' = 16c + g2 holds the 4
        # consecutive node rows i = 16*g2 + 4c + q (q=0..3); free = agg
        # (q,h) then denom (q,h).  src [1, 16384] (partition 32c) and dst
        # [16, 1024] flatten to the same element order, so a plain DMA
        # repacks partitions.  Alternate HWDGE engines for queue overlap.
        pk_ad = const.tile([64, 1024], F32)
        for c in range(4):
            eng = nc.scalar if (ACT_DMA and c % 2 == 1) else nc.sync
            eng.dma_start(pk_ad[16 * c:16 * (c + 1), :],
                          stage_ad[32 * c:32 * c + 1, :])
        # Ux into the same [g', (q, h)] layout via a DRAM round-trip (DRAM
        # APs allow the partition permutation; SBUF ones do not).
        dram = ctx.enter_context(tc.tile_pool(name="dram", bufs=1,
                                              space="DRAM"))
        ux_dram = dram.tile([N, H], F32)
        nc.sync.dma_start(ux_dram.rearrange("(b i) h -> i b h", i=128), ux_sb)
        pk_u = const.tile([64, 512], F32)
        (nc.scalar if ACT_DMA else nc.sync).dma_start(
            pk_u, ux_dram.rearrange("(g2 c q) h -> c g2 (q h)", c=4, q=4))

        rec = const.tile([64, 512], F32)
        if RECIP_APPROX:
            nc.vector.reciprocal_approx_fast(rec, pk_ad[:, 512:1024])
        else:
            nc.vector.reciprocal(rec, pk_ad[:, 512:1024])
        res = const.tile([64, 512], F32)
        nc.vector.tensor_mul(res, pk_ad[:, 0:512], rec)
        nc.vector.tensor_add(res, res, pk_u)
        nc.sync.dma_start(
            out_d.rearrange("(g2 c q) h -> c g2 (q h)", c=4, q=4), res)

    nc.compile()
    return nc


_NC_CACHE = None


def _get_program():
    global _NC_CACHE
    if _NC_CACHE is None:
        _NC_CACHE = build_program()
    return _NC_CACHE


def kernel(**inputs: np.ndarray) -> np.ndarray:
    x = np.ascontiguousarray(np.asarray(inputs["x"], dtype=np.float32))
    gate = np.ascontiguousarray(
        np.asarray(inputs["edge_gate"], dtype=np.float32))
    u_w = np.ascontiguousarray(np.asarray(inputs["U_w"], dtype=np.float32))
    u_b = np.ascontiguousarray(np.asarray(inputs["U_b"], dtype=np.float32))
    v_w = np.ascontiguousarray(np.asarray(inputs["V_w"], dtype=np.float32))
    v_b = np.ascontiguousarray(np.asarray(inputs["V_b"], dtype=np.float32))

    nc = _get_program()
    in_maps = [
        {
            "x": x[c],
            "gate": gate[c],
            "U_w": u_w,
            "U_b": u_b,
            "V_w": v_w,
            "V_b": v_b,
        }
        for c in range(NCORES)
    ]
    res = bass_utils.run_bass_kernel_spmd(
        nc, in_maps, core_ids=list(range(NCORES)))
    return np.stack([res.results[c]["out"] for c in range(NCORES)], axis=0)



# revision 8
# speedup vs baseline: 1.0606x; 1.0606x over previous
"""Trainium2 Bass kernel for nn_NodeFeatures (GNN message passing).

Math (per batch b):
    Ux  = (x @ U_w.T + U_b) * 0.5                      # (N, H)
    Vx  = (x @ V_w.T + V_b) * 0.5                      # (N, H)
    agg[i,h]   = sum_j gate[i,j,h] * Vx[j,h]
    denom[i,h] = 1e-20 + sum_j gate[i,j,h]
    out = Ux + agg / denom

Sharding: data-parallel over batch B=8 across the 8 NeuronCores (one batch
per core); H x H weights replicated.

Per-core plan (memory-bound; DMA_ENGINES transfer of the bf16-cast gate
stream ~47us is the floor):
  - gate tiles [j=128, i16=16, h=128] DMA'd with f32->bf16 cast (SWDGE).
  - DVE: prod = gate * Vx (bf16 2x-mode, one pass over the gate).
  - PE: per node i, a stationary-weights matmul reduces over j:
    ldweights(lhsT = prod[:, i, :] [j,h]) + matmul(rhs = ones[128,1])
    -> one PSUM column [h, 1] per (node, j-half); same for gate -> denom.
    Ldweights is free in the PE cost model and each matmul streams a single
    column, so the whole reduction is a few us instead of a 55us rhs-stream.
  - Accumulators live transposed: pa/pd [h=128, (jt, node)] PSUM columns.
  - Epilogue (all in transposed [h, node] space): add j-halves, reciprocal,
    UxT = U_w @ x^T via one matmul (+ per-partition bias), resT = UxT +
    aggT * recip(denT); two PE transposes restore [node, h]; one contiguous
    DMA writes the output.
"""

import sys

import numpy as np

try:
    import concourse.bass as bass  # noqa: F401
except ImportError:  # pragma: no cover
    sys.path.insert(0, "/opt/trn_rl_repo")

from contextlib import ExitStack

import concourse.bacc as bacc
import concourse.mybir as mybir
import concourse.tile as tile
from concourse import bass_utils
from concourse.masks import make_identity

F32 = mybir.dt.float32
BF16 = mybir.dt.bfloat16

B, N, H = 8, 256, 128
NCORES = 8
I16 = 16              # i values per gate tile
G2 = N // I16         # 16 tiles per j-half
JT = N // 128         # 2 j-halves

GATE_DT = BF16        # gate stream dtype (cast during SWDGE DMA)
DMA_AHEAD = 5         # gate tiles the DMA stream may run ahead


def build_program():
    """Build the per-core Bass program (identical on all 8 cores)."""
    nc = bacc.Bacc("TRN2", target_bir_lowering=False, debug=False,
                   num_devices=NCORES)

    x_d = nc.dram_tensor("x", [N, H], F32, kind="ExternalInput").ap()
    g_d = nc.dram_tensor("gate", [N, N, H], F32, kind="ExternalInput").ap()
    uw_d = nc.dram_tensor("U_w", [H, H], F32, kind="ExternalInput").ap()
    ub_d = nc.dram_tensor("U_b", [H], F32, kind="ExternalInput").ap()
    vw_d = nc.dram_tensor("V_w", [H, H], F32, kind="ExternalInput").ap()
    vb_d = nc.dram_tensor("V_b", [H], F32, kind="ExternalInput").ap()
    out_d = nc.dram_tensor("out", [N, H], F32, kind="ExternalOutput").ap()

    with tile.TileContext(nc) as tc, ExitStack() as ctx:
        const = ctx.enter_context(tc.tile_pool(name="const", bufs=1))

        # ---- small input loads -------------------------------------------
        x_sb = const.tile([128, 2, H], F32)           # [i_in_block, blk, h]
        nc.sync.dma_start(x_sb, x_d.rearrange("(b i) h -> i b h", i=128))
        uw_sb = const.tile([H, H], F32)
        nc.sync.dma_start(uw_sb, uw_d)
        vw_sb = const.tile([H, H], F32)
        nc.sync.dma_start(vw_sb, vw_d)
        # U_b as a per-partition column (bias lives on the partition dim in
        # transposed space); V_b broadcast along partitions (free dim h).
        ub_col = const.tile([128, 1], F32)
        nc.sync.dma_start(ub_col, ub_d[:, None])
        nc.vector.tensor_scalar_mul(ub_col, ub_col, 0.5)
        bv_half = const.tile([128, H], F32)
        nc.sync.dma_start(bv_half, vb_d[None, :].to_broadcast((128, H)))
        nc.vector.tensor_scalar_mul(bv_half, bv_half, 0.5)

        ident = const.tile([128, 128], F32)
        make_identity(nc, ident)
        ones_col = const.tile([128, 1], GATE_DT)
        nc.gpsimd.memset(ones_col, 1.0)

        # ---- setup: transposes, Vx (node-major) and UxT (h-major) --------
        xT = const.tile([H, N], F32)                  # [h, i]
        uwT = const.tile([H, H], F32)                 # [h, k]
        vwT = const.tile([H, H], F32)
        vx0 = const.tile([128, 1, 1, H], BF16)        # [j, 1, 1, h], j-half 0
        vx1 = const.tile([128, 1, 1, H], BF16)
        vx = [vx0, vx1]
        uxT = const.tile([128, N], F32)               # [k, i] = Ux transposed

        with tc.tile_pool(name="spsum", bufs=2, space="PSUM") as spsum:
            for blk in range(2):
                pt = spsum.tile([128, 128], F32, tag="tr")
                nc.tensor.transpose(pt, x_sb[:, blk, :], ident)
                nc.scalar.copy(xT[:, blk * 128:(blk + 1) * 128], pt)
            ptu = spsum.tile([128, 128], F32, tag="tr")
            nc.tensor.transpose(ptu, uw_sb, ident)
            nc.scalar.copy(uwT, ptu)
            ptv = spsum.tile([128, 128], F32, tag="tr")
            nc.tensor.transpose(ptv, vw_sb, ident)
            nc.scalar.copy(vwT, ptv)

            for blk in range(2):
                lhs = xT[:, blk * 128:(blk + 1) * 128]
                pv = spsum.tile([128, 128], F32, tag="mm")
                nc.tensor.matmul(pv, lhsT=lhs, rhs=vwT, start=True, stop=True)
                # vx = psum*0.5 + 0.5*V_b  (cast to bf16 on write)
                nc.vector.scalar_tensor_tensor(
                    vx[blk][:, 0, 0, :], pv, 0.5, bv_half,
                    op0=mybir.AluOpType.mult, op1=mybir.AluOpType.add)
            # UxT[k, i] = sum_h U_w[k, h] * x[i, h]  (one 256-col matmul)
            pu = spsum.tile([128, N], F32, tag="mmu")
            nc.tensor.matmul(pu, lhsT=uwT, rhs=xT, start=True, stop=True)
            # uxT = 0.5*psum + 0.5*U_b[k]  (per-partition bias column)
            nc.vector.scalar_tensor_tensor(
                uxT, pu, 0.5, ub_col.to_broadcast((128, N)),
                op0=mybir.AluOpType.mult, op1=mybir.AluOpType.add)

        # ---- main stream over gate ---------------------------------------
        # DRAM view: [g2, jt, j, i16, h]
        gv = g_d.rearrange("(g i) (t j) h -> g t j i h", i=I16, j=128)

        # Persistent PSUM accumulators, transposed: [h, (g2, i)]; the two
        # j-halves accumulate into the same column via matmul start/stop.
        acc_pool = ctx.enter_context(
            tc.tile_pool(name="acc", bufs=1, space="PSUM"))
        pa = acc_pool.tile([128, G2, I16], F32, tag="agg")
        pd = acc_pool.tile([128, G2, I16], F32, tag="den")

        gate_pool = ctx.enter_context(
            tc.tile_pool(name="gate", bufs=DMA_AHEAD))
        prod_pool = ctx.enter_context(tc.tile_pool(name="prod", bufs=4))

        def issue_gate_dma(g2, jt):
            gt = gate_pool.tile([128, I16, H], GATE_DT, tag="g",
                                name=f"gt_{g2}_{jt}")
            nc.gpsimd.dma_start(gt, gv[g2, jt])
            return gt

        # front-run the first tiles' DMAs so the stream starts at t=0
        pre = {}
        for g2 in range(2):
            for jt in range(JT):
                pre[(g2, jt)] = issue_gate_dma(g2, jt)

        for g2 in range(G2):
            gts, prs = [], []
            for jt in range(JT):
                gt = pre.pop((g2, jt), None)
                if gt is None:
                    gt = issue_gate_dma(g2, jt)
                pr = prod_pool.tile([128, I16, H], GATE_DT, tag="p",
                                    name=f"pr_{g2}_{jt}")
                nc.vector.tensor_mul(
                    pr, gt, vx[jt][:, 0].to_broadcast((128, I16, H)))
                gts.append(gt)
                prs.append(pr)
            # Accumulation groups must be back-to-back matmuls (CoreSim's
            # PSUM model does not support interleaved groups), so emit both
            # j-halves of a column consecutively.
            for i in range(I16):
                nc.tensor.matmul(pd[:, g2, i:i + 1], lhsT=gts[0][:, i, :],
                                 rhs=ones_col, start=True, stop=False)
                nc.tensor.matmul(pd[:, g2, i:i + 1], lhsT=gts[1][:, i, :],
                                 rhs=ones_col, start=False, stop=True)
                nc.tensor.matmul(pa[:, g2, i:i + 1], lhsT=prs[0][:, i, :],
                                 rhs=ones_col, start=True, stop=False)
                nc.tensor.matmul(pa[:, g2, i:i + 1], lhsT=prs[1][:, i, :],
                                 rhs=ones_col, start=False, stop=True)

        # ---- epilogue (transposed space: [h, node]) ----------------------
        # Each DVE op reads at most one PSUM operand.
        rec = const.tile([128, N], F32)
        nc.vector.reciprocal(rec, pd.rearrange("p g i -> p (g i)"))
        resT = const.tile([128, N], F32)
        nc.vector.tensor_mul(resT, pa.rearrange("p g i -> p (g i)"), rec)
        nc.vector.tensor_add(resT, resT, uxT)

        # transpose back to [node, h] and store
        res = const.tile([128, 2, H], F32)
        with tc.tile_pool(name="epsum", bufs=2, space="PSUM") as epsum:
            for blk in range(2):
                pt = epsum.tile([128, 128], F32, tag="etr")
                nc.tensor.transpose(
                    pt, resT[:, blk * 128:(blk + 1) * 128], ident)
                nc.scalar.copy(res[:, blk, :], pt)
        nc.sync.dma_start(out_d.rearrange("(b i) h -> i b h", i=128), res)

    nc.compile()
    return nc


_NC_CACHE = None


def _get_program():
    global _NC_CACHE
    if _NC_CACHE is None:
        _NC_CACHE = build_program()
    return _NC_CACHE


def kernel(**inputs: np.ndarray) -> np.ndarray:
    x = np.ascontiguousarray(np.asarray(inputs["x"], dtype=np.float32))
    gate = np.ascontiguousarray(
        np.asarray(inputs["edge_gate"], dtype=np.float32))
    u_w = np.ascontiguousarray(np.asarray(inputs["U_w"], dtype=np.float32))
    u_b = np.ascontiguousarray(np.asarray(inputs["U_b"], dtype=np.float32))
    v_w = np.ascontiguousarray(np.asarray(inputs["V_w"], dtype=np.float32))
    v_b = np.ascontiguousarray(np.asarray(inputs["V_b"], dtype=np.float32))

    nc = _get_program()
    in_maps = [
        {
            "x": x[c],
            "gate": gate[c],
            "U_w": u_w,
            "U_b": u_b,
            "V_w": v_w,
            "V_b": v_b,
        }
        for c in range(NCORES)
    ]
    res = bass_utils.run_bass_kernel_spmd(
        nc, in_maps, core_ids=list(range(NCORES)))
    return np.stack([res.results[c]["out"] for c in range(NCORES)], axis=0)


# revision 9
# speedup vs baseline: 1.6276x; 1.5346x over previous
"""Trainium2 Bass kernel for nn_NodeFeatures (GNN message passing).

Math (per batch b):
    Ux  = (x @ U_w.T + U_b) * 0.5                      # (N, H)
    Vx  = (x @ V_w.T + V_b) * 0.5                      # (N, H)
    agg[i,h]   = sum_j gate[i,j,h] * Vx[j,h]
    denom[i,h] = 1e-20 + sum_j gate[i,j,h]
    out = Ux + agg / denom

Sharding: data-parallel over batch B=8 across the 8 NeuronCores (one batch
per core); H x H weights replicated.

Per-core plan (memory-bound; DMA_ENGINES transfer of the bf16-cast gate
stream ~47us is the floor):
  - gate tiles [p=128, i16, (s h)=256] bf16 (SWDGE cast DMA): partition p
    holds the j-row PAIR j = 2p+s for node block i.  Merging (s, h) makes
    the innermost contiguous run 512B on the bf16 side (1024B on the DRAM
    side), avoiding the 2x descriptor-latency penalty for sub-512B runs.
  - DVE: prod = gate * Vx (bf16 2x-mode, one pass over the gate), with Vx
    re-laid-out to the same [p, (s h)] pairing via a DRAM round-trip.
  - PE: per node i, stationary-weights matmuls reduce over j:
    ldweights(lhsT = prod[:, i, 128s:]) + matmul(rhs = ones[128, 1])
    -> one PSUM column [h, 1] per node; s=0/1 accumulate back-to-back via
    start/stop.  Same for gate -> denom.  Ldweights is free in the PE cost
    model and each matmul streams a single column, so the whole reduction
    is a few us instead of a 55us rhs-stream.
  - Accumulators live transposed: pa/pd [h=128, node] PSUM columns.
  - Epilogue (in transposed [h, node] space): reciprocal, UxT = U_w @ x^T
    via one matmul (+ per-partition bias column), resT = UxT + aggT * rec;
    two PE transposes restore [node, h]; one contiguous DMA writes out.
"""

import sys

import numpy as np

try:
    import concourse.bass as bass  # noqa: F401
except ImportError:  # pragma: no cover
    sys.path.insert(0, "/opt/trn_rl_repo")

from contextlib import ExitStack

import concourse.bacc as bacc
import concourse.mybir as mybir
import concourse.tile as tile
from concourse import bass_utils
from concourse.masks import make_identity

F32 = mybir.dt.float32
BF16 = mybir.dt.bfloat16

B, N, H = 8, 256, 128
NCORES = 8
I16 = 16              # nodes per gate tile
G = N // I16          # 16 tiles, each covering all j for I16 nodes
SH = 2 * H            # merged (s, h) free run: j-pair per partition

GATE_DT = BF16
DMA_AHEAD = 3         # gate tiles the DMA stream may run ahead


def build_program():
    """Build the per-core Bass program (identical on all 8 cores)."""
    nc = bacc.Bacc("TRN2", target_bir_lowering=False, debug=False,
                   num_devices=NCORES)

    x_d = nc.dram_tensor("x", [N, H], F32, kind="ExternalInput").ap()
    g_d = nc.dram_tensor("gate", [N, N, H], F32, kind="ExternalInput").ap()
    uw_d = nc.dram_tensor("U_w", [H, H], F32, kind="ExternalInput").ap()
    ub_d = nc.dram_tensor("U_b", [H], F32, kind="ExternalInput").ap()
    vw_d = nc.dram_tensor("V_w", [H, H], F32, kind="ExternalInput").ap()
    vb_d = nc.dram_tensor("V_b", [H], F32, kind="ExternalInput").ap()
    out_d = nc.dram_tensor("out", [N, H], F32, kind="ExternalOutput").ap()

    with tile.TileContext(nc) as tc, ExitStack() as ctx:
        const = ctx.enter_context(tc.tile_pool(name="const", bufs=1))

        # ---- small input loads -------------------------------------------
        x_sb = const.tile([128, 2, H], F32)           # [i_in_block, blk, h]
        nc.sync.dma_start(x_sb, x_d.rearrange("(b i) h -> i b h", i=128))
        uw_sb = const.tile([H, H], F32)
        nc.sync.dma_start(uw_sb, uw_d)
        vw_sb = const.tile([H, H], F32)
        nc.sync.dma_start(vw_sb, vw_d)
        # U_b as a per-partition column (bias lives on the partition dim in
        # transposed space); V_b broadcast along partitions (free dim h).
        ub_col = const.tile([128, 1], F32)
        nc.sync.dma_start(ub_col, ub_d[:, None])
        nc.vector.tensor_scalar_mul(ub_col, ub_col, 0.5)
        bv_half = const.tile([128, H], F32)
        nc.sync.dma_start(bv_half, vb_d[None, :].to_broadcast((128, H)))
        nc.vector.tensor_scalar_mul(bv_half, bv_half, 0.5)

        ident = const.tile([128, 128], F32)
        make_identity(nc, ident)
        ones_col = const.tile([128, 1], GATE_DT)
        nc.gpsimd.memset(ones_col, 1.0)

        # ---- setup: transposes, Vx (j-pair layout) and UxT (h-major) -----
        xT = const.tile([H, N], F32)                  # [h, i]
        uwT = const.tile([H, H], F32)                 # [h, k]
        vwT = const.tile([H, H], F32)
        uxT = const.tile([128, N], F32)               # [k, i] = Ux transposed

        dram = ctx.enter_context(tc.tile_pool(name="dram", bufs=1,
                                              space="DRAM"))
        vx_dram = dram.tile([N, H], F32)

        with tc.tile_pool(name="spsum", bufs=2, space="PSUM") as spsum:
            for blk in range(2):
                pt = spsum.tile([128, 128], F32, tag="tr")
                nc.tensor.transpose(pt, x_sb[:, blk, :], ident)
                nc.scalar.copy(xT[:, blk * 128:(blk + 1) * 128], pt)
            ptu = spsum.tile([128, 128], F32, tag="tr")
            nc.tensor.transpose(ptu, uw_sb, ident)
            nc.scalar.copy(uwT, ptu)
            ptv = spsum.tile([128, 128], F32, tag="tr")
            nc.tensor.transpose(ptv, vw_sb, ident)
            nc.scalar.copy(vwT, ptv)

            for blk in range(2):
                lhs = xT[:, blk * 128:(blk + 1) * 128]
                pv = spsum.tile([128, 128], F32, tag="mm")
                nc.tensor.matmul(pv, lhsT=lhs, rhs=vwT, start=True, stop=True)
                vxb = const.tile([128, 128], F32, name=f"vx{blk}")
                # vx = psum*0.5 + 0.5*V_b
                nc.vector.scalar_tensor_tensor(
                    vxb, pv, 0.5, bv_half,
                    op0=mybir.AluOpType.mult, op1=mybir.AluOpType.add)
                # round-trip through DRAM to re-lay-out to j pairs
                nc.sync.dma_start(
                    vx_dram.rearrange("(b i) h -> i b h", i=128)[:, blk, :],
                    vxb)
            # UxT[k, i] = sum_h U_w[k, h] * x[i, h]  (one 256-col matmul)
            pu = spsum.tile([128, N], F32, tag="mmu")
            nc.tensor.matmul(pu, lhsT=uwT, rhs=xT, start=True, stop=True)
            # uxT = 0.5*psum + 0.5*U_b[k]  (per-partition bias column)
            nc.vector.scalar_tensor_tensor(
                uxT, pu, 0.5, ub_col.to_broadcast((128, N)),
                op0=mybir.AluOpType.mult, op1=mybir.AluOpType.add)

        # Vx in j-pair layout: vx_pair[p, s*H + h] = Vx[2p + s, h]
        vx_pair = const.tile([128, SH], GATE_DT)
        nc.gpsimd.dma_start(vx_pair,
                            vx_dram.rearrange("(p s) h -> p (s h)", s=2))

        # ---- main stream over gate ---------------------------------------
        # DRAM view: [g, p, i, (s h)]; per (g, p, i) the run is 1024B contig.
        gv = g_d.rearrange("(g i) (p s) h -> g p i (s h)", i=I16, s=2)

        # Persistent PSUM accumulators, transposed: [h, (g, i)]
        acc_pool = ctx.enter_context(
            tc.tile_pool(name="acc", bufs=1, space="PSUM"))
        pa = acc_pool.tile([128, G, I16], F32, tag="agg")
        pd = acc_pool.tile([128, G, I16], F32, tag="den")

        gate_pool = ctx.enter_context(
            tc.tile_pool(name="gate", bufs=DMA_AHEAD))
        prod_pool = ctx.enter_context(tc.tile_pool(name="prod", bufs=2))

        def issue_gate_dma(g):
            gt = gate_pool.tile([128, I16, SH], GATE_DT, tag="g",
                                name=f"gt_{g}")
            nc.gpsimd.dma_start(gt, gv[g])
            return gt

        pre = {g: issue_gate_dma(g) for g in range(2)}

        for g in range(G):
            gt = pre.pop(g, None)
            if gt is None:
                gt = issue_gate_dma(g)
            pr = prod_pool.tile([128, I16, SH], GATE_DT, tag="p",
                                name=f"pr_{g}")
            nc.vector.tensor_mul(
                pr, gt, vx_pair[:, None, :].to_broadcast((128, I16, SH)))
            for i in range(I16):
                # s = 0/1 sub-rows accumulate back-to-back (CoreSim's PSUM
                # model does not support interleaved accumulation groups)
                nc.tensor.matmul(pd[:, g, i:i + 1], lhsT=gt[:, i, 0:H],
                                 rhs=ones_col, start=True, stop=False)
                nc.tensor.matmul(pd[:, g, i:i + 1], lhsT=gt[:, i, H:SH],
                                 rhs=ones_col, start=False, stop=True)
                nc.tensor.matmul(pa[:, g, i:i + 1], lhsT=pr[:, i, 0:H],
                                 rhs=ones_col, start=True, stop=False)
                nc.tensor.matmul(pa[:, g, i:i + 1], lhsT=pr[:, i, H:SH],
                                 rhs=ones_col, start=False, stop=True)

        # ---- epilogue (transposed space: [h, node]) ----------------------
        # Each DVE op reads at most one PSUM operand.
        rec = const.tile([128, N], F32)
        nc.vector.reciprocal(rec, pd.rearrange("p g i -> p (g i)"))
        resT = const.tile([128, N], F32)
        nc.vector.tensor_mul(resT, pa.rearrange("p g i -> p (g i)"), rec)
        nc.vector.tensor_add(resT, resT, uxT)

        # transpose back to [node, h] and store
        res = const.tile([128, 2, H], F32)
        with tc.tile_pool(name="epsum", bufs=2, space="PSUM") as epsum:
            for blk in range(2):
                pt = epsum.tile([128, 128], F32, tag="etr")
                nc.tensor.transpose(
                    pt, resT[:, blk * 128:(blk + 1) * 128], ident)
                nc.scalar.copy(res[:, blk, :], pt)
        nc.sync.dma_start(out_d.rearrange("(b i) h -> i b h", i=128), res)

    nc.compile()
    return nc


_NC_CACHE = None


def _get_program():
    global _NC_CACHE
    if _NC_CACHE is None:
        _NC_CACHE = build_program()
    return _NC_CACHE


def kernel(**inputs: np.ndarray) -> np.ndarray:
    x = np.ascontiguousarray(np.asarray(inputs["x"], dtype=np.float32))
    gate = np.ascontiguousarray(
        np.asarray(inputs["edge_gate"], dtype=np.float32))
    u_w = np.ascontiguousarray(np.asarray(inputs["U_w"], dtype=np.float32))
    u_b = np.ascontiguousarray(np.asarray(inputs["U_b"], dtype=np.float32))
    v_w = np.ascontiguousarray(np.asarray(inputs["V_w"], dtype=np.float32))
    v_b = np.ascontiguousarray(np.asarray(inputs["V_b"], dtype=np.float32))

    nc = _get_program()
    in_maps = [
        {
            "x": x[c],
            "gate": gate[c],
            "U_w": u_w,
            "U_b": u_b,
            "V_w": v_w,
            "V_b": v_b,
        }
        for c in range(NCORES)
    ]
    res = bass_utils.run_bass_kernel_spmd(
        nc, in_maps, core_ids=list(range(NCORES)))
    return np.stack([res.results[c]["out"] for c in range(NCORES)], axis=0)


# revision 11
# speedup vs baseline: 1.9100x; 1.1735x over previous
"""Trainium2 Bass kernel for nn_NodeFeatures (GNN message passing).

Math (per batch b):
    Ux  = (x @ U_w.T + U_b) * 0.5                      # (N, H)
    Vx  = (x @ V_w.T + V_b) * 0.5                      # (N, H)
    agg[i,h]   = sum_j gate[i,j,h] * Vx[j,h]
    denom[i,h] = 1e-20 + sum_j gate[i,j,h]
    out = Ux + agg / denom

Sharding: data-parallel over batch B=8 across the 8 NeuronCores (one batch
per core); H x H weights replicated.

Per-core plan (memory-bound; DMA_ENGINES transfer of the bf16-cast gate
stream ~47us is the floor):
  - gate tiles [p=128, i16, (s h)=256] bf16 (SWDGE cast DMA): partition p
    holds the j-row PAIR j = 2p+s for node block i.  Merging (s, h) makes
    the innermost contiguous run 512B on the bf16 side (1024B on the DRAM
    side), avoiding the 2x descriptor-latency penalty for sub-512B runs.
  - DVE: prod = gate * Vx (bf16 2x-mode, one pass over the gate).  Vx is
    produced directly in the same [p, (s h)] pairing by two matmuls whose
    stationary lhsT is a stride-2 node slice of x^T (no DRAM round-trip).
  - PE: per node i, stationary-weights matmuls reduce over j:
    ldweights(lhsT = prod[:, i, 128s:]) + matmul(rhs = ones[128, 1])
    -> one PSUM column [h, 1] per node; s=0/1 accumulate back-to-back via
    start/stop.  Same for gate -> denom.  Ldweights is free in the PE cost
    model and each matmul streams a single column, so the whole reduction
    is a few us instead of a 55us rhs-stream.
  - Accumulators live transposed: pa/pd [h=128, node] PSUM columns.
  - Epilogue is pipelined: after every 2 tiles a 32-column fragment runs
    (reciprocal, *agg, +UxT) on DVE slack; each 128-node block transposes
    (PE) and streams out as soon as its fragments are done, so only the
    last fragment + one transpose + one small DMA trail the gate stream.
"""

import sys

import numpy as np

try:
    import concourse.bass as bass  # noqa: F401
except ImportError:  # pragma: no cover
    sys.path.insert(0, "/opt/trn_rl_repo")

from contextlib import ExitStack

import concourse.bacc as bacc
import concourse.mybir as mybir
import concourse.tile as tile
from concourse import bass_utils
from concourse.masks import make_identity

F32 = mybir.dt.float32
BF16 = mybir.dt.bfloat16

B, N, H = 8, 256, 128
NCORES = 8
I16 = 16              # nodes per gate tile
G = N // I16          # 16 tiles, each covering all j for I16 nodes
SH = 2 * H            # merged (s, h) free run: j-pair per partition

GATE_DT = BF16
DMA_AHEAD = 4         # gate tiles the DMA stream may run ahead
FRAG = 2              # tiles per epilogue fragment


def build_program():
    """Build the per-core Bass program (identical on all 8 cores)."""
    nc = bacc.Bacc("TRN2", target_bir_lowering=False, debug=False,
                   num_devices=NCORES)

    x_d = nc.dram_tensor("x", [N, H], F32, kind="ExternalInput").ap()
    g_d = nc.dram_tensor("gate", [N, N, H], F32, kind="ExternalInput").ap()
    uw_d = nc.dram_tensor("U_w", [H, H], F32, kind="ExternalInput").ap()
    ub_d = nc.dram_tensor("U_b", [H], F32, kind="ExternalInput").ap()
    vw_d = nc.dram_tensor("V_w", [H, H], F32, kind="ExternalInput").ap()
    vb_d = nc.dram_tensor("V_b", [H], F32, kind="ExternalInput").ap()
    out_d = nc.dram_tensor("out", [N, H], F32, kind="ExternalOutput").ap()

    ov = out_d.rearrange("(b i) h -> i b h", i=128)

    with tile.TileContext(nc) as tc, ExitStack() as ctx:
        const = ctx.enter_context(tc.tile_pool(name="const", bufs=1))
        gate_pool = ctx.enter_context(
            tc.tile_pool(name="gate", bufs=DMA_AHEAD))
        prod_pool = ctx.enter_context(tc.tile_pool(name="prod", bufs=2))
        acc_pool = ctx.enter_context(
            tc.tile_pool(name="acc", bufs=1, space="PSUM"))

        # ---- Pool-engine setup first (so gate desc-gen isn't delayed) ----
        ident = const.tile([128, 128], F32)
        make_identity(nc, ident)
        ones_col = const.tile([128, 1], GATE_DT)
        nc.gpsimd.memset(ones_col, 1.0)

        # ---- gate stream: pre-issue --------------------------------------
        # DRAM view: [g, p, i, (s h)]; per (g, p, i) the run is 1024B contig.
        gv = g_d.rearrange("(g i) (p s) h -> g p i (s h)", i=I16, s=2)

        def issue_gate_dma(g):
            gt = gate_pool.tile([128, I16, SH], GATE_DT, tag="g",
                                name=f"gt_{g}")
            nc.gpsimd.dma_start(gt, gv[g])
            return gt

        pre = {g: issue_gate_dma(g) for g in range(DMA_AHEAD)}

        # ---- small input loads -------------------------------------------
        x_sb = const.tile([128, 2, H], F32)           # [i_in_block, blk, h]
        nc.sync.dma_start(x_sb, x_d.rearrange("(b i) h -> i b h", i=128))
        uw_sb = const.tile([H, H], F32)
        nc.sync.dma_start(uw_sb, uw_d)
        vw_sb = const.tile([H, H], F32)
        nc.sync.dma_start(vw_sb, vw_d)
        # U_b as a per-partition column (bias lives on the partition dim in
        # transposed space); V_b broadcast along partitions (free dim h).
        ub_col = const.tile([128, 1], F32)
        nc.sync.dma_start(ub_col, ub_d[:, None])
        nc.vector.tensor_scalar_mul(ub_col, ub_col, 0.5)
        bv_half = const.tile([128, H], F32)
        nc.sync.dma_start(bv_half, vb_d[None, :].to_broadcast((128, H)))
        nc.vector.tensor_scalar_mul(bv_half, bv_half, 0.5)

        # ---- setup: transposes, Vx (j-pair layout) and UxT (h-major) -----
        xT = const.tile([H, N], F32)                  # [h, i]
        uwT = const.tile([H, H], F32)                 # [h, k]
        vwT = const.tile([H, H], F32)
        uxT = const.tile([128, N], F32)               # [k, i] = Ux transposed
        # vx_pair[p, s*H + h] = Vx[2p + s, h]
        vx_pair = const.tile([128, 2, H], GATE_DT)
        xTs = xT.rearrange("h (i s) -> h s i", s=2)   # stride-2 node slices

        with tc.tile_pool(name="spsum", bufs=2, space="PSUM") as spsum:
            for blk in range(2):
                pt = spsum.tile([128, 128], F32, tag="tr")
                nc.tensor.transpose(pt, x_sb[:, blk, :], ident)
                nc.scalar.copy(xT[:, blk * 128:(blk + 1) * 128], pt)
            ptv = spsum.tile([128, 128], F32, tag="tr")
            nc.tensor.transpose(ptv, vw_sb, ident)
            nc.scalar.copy(vwT, ptv)
            ptu = spsum.tile([128, 128], F32, tag="tr")
            nc.tensor.transpose(ptu, uw_sb, ident)
            nc.scalar.copy(uwT, ptu)

            for s in range(2):
                # out partition p = node 2p+s: lhsT free dim strided by 2
                pv = spsum.tile([128, 128], F32, tag="mm")
                nc.tensor.matmul(pv, lhsT=xTs[:, s, :], rhs=vwT,
                                 start=True, stop=True)
                nc.vector.scalar_tensor_tensor(
                    vx_pair[:, s, :], pv, 0.5, bv_half,
                    op0=mybir.AluOpType.mult, op1=mybir.AluOpType.add)
            # UxT[k, i] = sum_h U_w[k, h] * x[i, h]  (one 256-col matmul)
            pu = spsum.tile([128, N], F32, tag="mmu")
            nc.tensor.matmul(pu, lhsT=uwT, rhs=xT, start=True, stop=True)
            # uxT = 0.5*psum + 0.5*U_b[k]  (per-partition bias column)
            nc.vector.scalar_tensor_tensor(
                uxT, pu, 0.5, ub_col.to_broadcast((128, N)),
                op0=mybir.AluOpType.mult, op1=mybir.AluOpType.add)

        # ---- main stream over gate ---------------------------------------
        # Persistent PSUM accumulators, transposed: [h, (g, i)]
        pa = acc_pool.tile([128, G, I16], F32, tag="agg")
        pd = acc_pool.tile([128, G, I16], F32, tag="den")
        pav = pa.rearrange("p g i -> p (g i)")
        pdv = pd.rearrange("p g i -> p (g i)")

        rec = const.tile([128, N], F32)
        resT = const.tile([128, N], F32)
        res = const.tile([128, 2, H], F32)
        epsum = ctx.enter_context(
            tc.tile_pool(name="epsum", bufs=2, space="PSUM"))

        for g in range(G):
            gt = pre.pop(g, None)
            if gt is None:
                gt = issue_gate_dma(g)
            pr = prod_pool.tile([128, I16, SH], GATE_DT, tag="p",
                                name=f"pr_{g}")
            nc.vector.tensor_mul(
                pr, gt,
                vx_pair.rearrange("p s h -> p (s h)")[:, None, :]
                .to_broadcast((128, I16, SH)))
            for i in range(I16):
                # s = 0/1 sub-rows accumulate back-to-back (CoreSim's PSUM
                # model does not support interleaved accumulation groups)
                nc.tensor.matmul(pd[:, g, i:i + 1], lhsT=gt[:, i, 0:H],
                                 rhs=ones_col, start=True, stop=False)
                nc.tensor.matmul(pd[:, g, i:i + 1], lhsT=gt[:, i, H:SH],
                                 rhs=ones_col, start=False, stop=True)
                nc.tensor.matmul(pa[:, g, i:i + 1], lhsT=pr[:, i, 0:H],
                                 rhs=ones_col, start=True, stop=False)
                nc.tensor.matmul(pa[:, g, i:i + 1], lhsT=pr[:, i, H:SH],
                                 rhs=ones_col, start=False, stop=True)

            # ---- pipelined epilogue fragments on DVE slack ----------------
            if g % FRAG == FRAG - 1:
                lo, hi = (g + 1 - FRAG) * I16, (g + 1) * I16
                cols = slice(lo, hi)
                nc.vector.reciprocal(rec[:, cols], pdv[:, cols])
                nc.vector.tensor_mul(resT[:, cols], pav[:, cols],
                                     rec[:, cols])
                nc.vector.tensor_add(resT[:, cols], resT[:, cols],
                                     uxT[:, cols])
            # block of 128 nodes complete -> transpose + stream out
            if (g + 1) % (G // 2) == 0:
                blk = (g + 1) // (G // 2) - 1
                pt = epsum.tile([128, 128], F32, tag="etr")
                nc.tensor.transpose(
                    pt, resT[:, blk * 128:(blk + 1) * 128], ident)
                nc.scalar.copy(res[:, blk, :], pt)
                nc.sync.dma_start(ov[:, blk, :], res[:, blk, :])

    nc.compile()
    return nc


_NC_CACHE = None


def _get_program():
    global _NC_CACHE
    if _NC_CACHE is None:
        _NC_CACHE = build_program()
    return _NC_CACHE


def kernel(**inputs: np.ndarray) -> np.ndarray:
    x = np.ascontiguousarray(np.asarray(inputs["x"], dtype=np.float32))
    gate = np.ascontiguousarray(
        np.asarray(inputs["edge_gate"], dtype=np.float32))
    u_w = np.ascontiguousarray(np.asarray(inputs["U_w"], dtype=np.float32))
    u_b = np.ascontiguousarray(np.asarray(inputs["U_b"], dtype=np.float32))
    v_w = np.ascontiguousarray(np.asarray(inputs["V_w"], dtype=np.float32))
    v_b = np.ascontiguousarray(np.asarray(inputs["V_b"], dtype=np.float32))

    nc = _get_program()
    in_maps = [
        {
            "x": x[c],
            "gate": gate[c],
            "U_w": u_w,
            "U_b": u_b,
            "V_w": v_w,
            "V_b": v_b,
        }
        for c in range(NCORES)
    ]
    res = bass_utils.run_bass_kernel_spmd(
        nc, in_maps, core_ids=list(range(NCORES)))
    return np.stack([res.results[c]["out"] for c in range(NCORES)], axis=0)
